# revision 2
# baseline (speedup 1.0000x reference)
"""Fused multi-head attention block (QKV proj + softmax attention + out proj
+ LN + relu-residual + LN) for Trainium2, SPMD across 8 NeuronCores.

Problem shapes (hardcoded): B=2, NQ=NK=4096, D=256, H=8, DH=32.

Sharding: sequence-parallel over (batch, query-chunk): core c handles batch
c//4, query rows [1024*(c%4), 1024*(c%4+1)). Each core reads the K rows of
its batch and computes its query chunk end-to-end. No collectives.

Per-core kernel (matmuls in fp32r = full-rate rounded fp32):
  phase A: QpT/KpT = W.T-chunk @ {Q,K}T, Vp = KT-tiles @ WvT.
  phase B: per head-group g (4 heads), q-block qb (512), k-tile j (128):
    scores^T[k,q] via 4 row-packed (K=32) matmuls; exp split between
    ScalarE (exact) and a custom cubic-poly DVE op, running in parallel;
    attn@V + softmax denominator via col-packed matmuls accumulating in
    PSUM; normalize by approx-reciprocal of the denominator. K-projection
    bias is dropped (softmax shift invariance); V bias rides on the
    normalized output (softmax rows sum to 1).
  tail: out-proj matmuls + LN0 + (x+relu(x)) + LN1; rsqrt computed as
    exp(-0.5*ln(var+eps)) so every ACT call stays in one table set.
"""

import os

import numpy as np

import concourse.bass as bass
import concourse.mybir as mybir
import concourse.tile as tile
from concourse import bacc
from concourse.bass_utils import run_bass_kernel_spmd

F32 = mybir.dt.float32
F32R = mybir.dt.float32r
BF16 = mybir.dt.bfloat16
AF = mybir.ActivationFunctionType
ALU = mybir.AluOpType

B, NQ, NK = 2, 4096, 4096
D = 256
H = 8
DH = 32
LN_EPS = 1e-5
NCORES = 8
QC = (B * NQ) // NCORES  # 1024 query rows per core
SCALE = 1.0 / np.sqrt(np.float32(DH))
NJ = NK // 128  # 32 k-tiles

# every Nth k-tile, ScalarE also takes the "B" exp tile (engine balancing)
ACT_TAKES_B_EVERY = 4

_DVE_OPS = {}


def _register_dve_ops():
    """Runtime-register the custom DVE ops used by this kernel."""
    if _DVE_OPS:
        return _DVE_OPS
    import concourse.dve_ops as dve_ops
    from concourse.dve_spec import (
        C0, C1, C2, C3, Spec, Src0, _spill_c3_to_src1, lower, relu,
    )
    from concourse.dve_uop import DveOpSpec

    def _mk(name, spec, rd1_en):
        for op in dve_ops.OPS:
            if op.name == name:
                return op
        row = dve_ops._CUSTOM_DVE_ROW_BASE + len(dve_ops.OPS)
        shas = {}
        for ver in ("v3", "v4"):
            tmp = DveOpSpec(name=name, opcode=row, uops=lower(spec, ver=ver),
                            rd1_en=rd1_en)
            shas[ver] = tmp.sha(ver)
        op = dve_ops.DveOp(name, spec, subdim=False, uops_sha=shas)
        dve_ops.OPS.append(op)
        dve_ops.CUSTOM_DVE_SPECS[op.name] = op.spec
        dve_ops._SUB_OPCODE_FOR_NAME[op.name] = row
        return op

    # cubic exp: out = ((c3*x + c2)*x + c1)*x + c0, c3 rides in1 ([P,1])
    def _exp3_ref(in0, in1, c0, c1, c2):
        c3 = in1[:, :1]
        x = in0.astype(np.float32)
        return ((c3 * x + c2) * x + c1) * x + c0

    exp3 = _mk(
        "EXP3_ANT",
        Spec(
            body=_spill_c3_to_src1(((C3 * Src0 + C2) * Src0 + C1) * Src0 + C0),
            reference=_exp3_ref,
        ),
        rd1_en=True,
    )

    # LN relu-residual: t = (x - mu)*rs; out = t + relu(t)
    def _relu2_ref(in0, in1, c0, c1, c2):
        t = (in0.astype(np.float32) - c0) * c1
        return t + np.maximum(np.nan_to_num(t, nan=0.0), 0)

    _t = (Src0 - C0) * C1
    relu2 = _mk(
        "RELU2LN_ANT",
        Spec(body=_t + relu(_t), reference=_relu2_ref),
        rd1_en=False,
    )
    _DVE_OPS["exp3"] = exp3
    _DVE_OPS["relu2"] = relu2
    return _DVE_OPS


def _fit_exp_cubic(scale, hi_raw):
    """Chebyshev-node cubic fit of e^(scale*x) for x in [-hi_raw, hi_raw]
    (raw, unscaled scores). Returns (c0, c1, c2, c3)."""
    t = np.cos(np.linspace(0, np.pi, 20001))
    xc = hi_raw * t
    yc = np.exp(np.float64(scale) * xc)
    c = np.polyfit(xc, yc, 3)
    return tuple(float(v) for v in c[::-1])


def _build_kernel(trivial_affine, repeat=1):
    """Build the SPMD Bass program. trivial_affine: all biases zero, all LN
    gammas one, betas zero (true for this problem's setup_inputs)."""
    ops = _register_dve_ops()
    exp3, relu2 = ops["exp3"], ops["relu2"]
    c0, c1, c2, c3 = _fit_exp_cubic(SCALE, 4.6)

    nc = bacc.Bacc("TRN2", target_bir_lowering=False)

    # ---- dram i/o ----
    qT = nc.dram_tensor("qT", [D, QC], F32R, kind="ExternalInput")
    kT = nc.dram_tensor("kT", [D, NK], F32R, kind="ExternalInput")
    wqT = nc.dram_tensor("wqT", [D, D], F32R, kind="ExternalInput")
    wkT = nc.dram_tensor("wkT", [D, D], F32R, kind="ExternalInput")
    wvT = nc.dram_tensor("wvT", [D, D], F32R, kind="ExternalInput")
    woT = nc.dram_tensor("woT", [D, D], F32R, kind="ExternalInput")
    # vecsP[d, i]: per-partition-use vectors; col 0=bq, 1=bv
    vecsP = nc.dram_tensor("vecsP", [D, 2], F32, kind="ExternalInput")
    # vecsF[i, d]: free-dim-use vectors; row 0=bo 1=g0 2=beta0 3=g1 4=beta1
    vecsF = nc.dram_tensor("vecsF", [5, D], F32, kind="ExternalInput")
    out = nc.dram_tensor("out", [QC, D], F32, kind="ExternalOutput")

    with tile.TileContext(nc) as tc:
        with tc.tile_pool(name="sb", bufs=1) as sb:
            # ---- load inputs ----
            qt = [sb.tile([128, QC], F32R, tag=f"qt{i}", name=f"qt{i}") for i in range(2)]
            kt = [sb.tile([128, NK], F32R, tag=f"kt{i}", name=f"kt{i}") for i in range(2)]
            wqt = [sb.tile([128, D], F32R, tag=f"wqt{i}", name=f"wqt{i}") for i in range(2)]
            wkt = [sb.tile([128, D], F32R, tag=f"wkt{i}", name=f"wkt{i}") for i in range(2)]
            wvt = [sb.tile([128, D], F32R, tag=f"wvt{i}", name=f"wvt{i}") for i in range(2)]
            wot = [sb.tile([128, D], F32R, tag=f"wot{i}", name=f"wot{i}") for i in range(2)]
            ones32 = sb.tile([128, 32], BF16)
            c3t = sb.tile([128, 1], F32)
            epst = sb.tile([128, 1], F32)
            vp_ = [sb.tile([128, 2], F32, tag=f"vp_{i}", name=f"vp_{i}") for i in range(2)]
            vf_ = sb.tile([128, 5, D], F32) if not trivial_affine else None
            for i in range(2):
                nc.sync.dma_start(out=wqt[i], in_=wqT[128 * i : 128 * i + 128, :])
                nc.sync.dma_start(out=wkt[i], in_=wkT[128 * i : 128 * i + 128, :])
                nc.sync.dma_start(out=wvt[i], in_=wvT[128 * i : 128 * i + 128, :])
                nc.sync.dma_start(out=wot[i], in_=woT[128 * i : 128 * i + 128, :])
                nc.sync.dma_start(out=qt[i], in_=qT[128 * i : 128 * i + 128, :])
                nc.sync.dma_start(out=kt[i], in_=kT[128 * i : 128 * i + 128, :])
                nc.sync.dma_start(out=vp_[i], in_=vecsP[128 * i : 128 * i + 128, :])
            nc.vector.memset(ones32, 1.0)
            if vf_ is not None:
                nc.gpsimd.dma_start(
                    out=vf_, in_=vecsF[:, :].unsqueeze(0).broadcast_to([128, 5, D])
                )
            nc.vector.memset(c3t, c3)
            nc.vector.memset(epst, LN_EPS)

            import contextlib as _ctxlib
            _loop = tc.For_i(0, repeat) if repeat > 1 else _ctxlib.nullcontext()
            with _loop:

                # ---- phase A: projections ----
                qpt = [sb.tile([128, QC], BF16, tag=f"qpt{g}", name=f"qpt{g}") for g in range(2)]
                kpt = [sb.tile([128, NK], BF16, tag=f"kpt{g}", name=f"kpt{g}") for g in range(2)]
                vp = sb.tile([128, NJ, D], BF16)

                with tc.tile_pool(name="psA", bufs=2, space="PSUM") as psA:
                    # QpT: [dv-chunk g 128, q 1024]
                    for g in range(2):
                        qp_ps = psA.tile([128, QC], F32, tag="qp_ps")
                        for qb in range(2):
                            for dc in range(2):
                                nc.tensor.matmul(
                                    qp_ps[:, 512 * qb : 512 * qb + 512],
                                    wqt[dc][:, 128 * g : 128 * g + 128],
                                    qt[dc][:, 512 * qb : 512 * qb + 512],
                                    start=(dc == 0),
                                    stop=(dc == 1),
                                )
                        if trivial_affine:
                            nc.vector.tensor_copy(qpt[g], qp_ps[:, :])
                        else:
                            nc.vector.tensor_scalar(
                                out=qpt[g], in0=qp_ps[:, :],
                                scalar1=vp_[g][:, 0:1], scalar2=None, op0=ALU.add,
                            )
                    # KpT (K bias dropped: softmax-invariant per query)
                    for g in range(2):
                        for kb in range(8):
                            kp_ps = psA.tile([128, 512], F32, tag="kp_ps")
                            for dc in range(2):
                                nc.tensor.matmul(
                                    kp_ps[:, :],
                                    wkt[dc][:, 128 * g : 128 * g + 128],
                                    kt[dc][:, 512 * kb : 512 * kb + 512],
                                    start=(dc == 0),
                                    stop=(dc == 1),
                                )
                            nc.vector.tensor_copy(
                                kpt[g][:, 512 * kb : 512 * kb + 512], kp_ps[:, :]
                            )
                    # Vp: [k-tile 128, dv 256] (V bias folded post-attention)
                    for kt_i in range(NJ):
                        vps = psA.tile([128, D], F32, tag="vps")
                        for dc in range(2):
                            nc.tensor.matmul(
                                vps[:, :],
                                kt[dc][:, 128 * kt_i : 128 * kt_i + 128],
                                wvt[dc][:, :],
                                start=(dc == 0),
                                stop=(dc == 1),
                            )
                        nc.scalar.activation(
                            out=vp[:, kt_i, :], in_=vps[:, :], func=AF.Copy
                        )

                # ---- phase B: attention ----
                with (
                    tc.tile_pool(name="scp", bufs=3, space="PSUM") as scp,
                    tc.tile_pool(name="avp", bufs=1, space="PSUM") as avp,
                    tc.tile_pool(name="dnp", bufs=1, space="PSUM") as dnp,
                    tc.tile_pool(name="upool", bufs=4) as upool,
                    tc.tile_pool(name="tails", bufs=3) as tails,
                ):
                    attnT = [
                        sb.tile([128, QC], F32R, tag=f"attnT{g}", name=f"attnT{g}") for g in range(2)
                    ]
                    for qb in range(2):
                        for g in range(2):
                            av_ps = avp.tile([128, 512], F32, tag="av")
                            dn_ps = dnp.tile([128, 512], F32, tag="dn")
                            prev_u = None
                            prev_j = -1
                            for j in range(NJ + 1):
                                if j < NJ:
                                    st = [
                                        scp.tile([128, 1024], F32, tag="sc", name="sc")
                                        for _ in range(2)
                                    ]
                                    for hp in range(4):
                                        nc.tensor.matmul(
                                            st[hp // 2][
                                                :, 512 * (hp % 2) : 512 * (hp % 2) + 512
                                            ],
                                            kpt[g][
                                                32 * hp : 32 * hp + 32,
                                                128 * j : 128 * j + 128,
                                            ],
                                            qpt[g][
                                                32 * hp : 32 * hp + 32,
                                                512 * qb : 512 * qb + 512,
                                            ],
                                            start=True,
                                            stop=True,
                                            tile_position=(32 * hp, 0),
                                        )
                                    u = [
                                        upool.tile([128, 1024], BF16, tag="u", name="u")
                                        for _ in range(2)
                                    ]
                                    nc.scalar.activation(
                                        out=u[0], in_=st[0][:, :], func=AF.Exp,
                                        scale=float(SCALE),
                                    )
                                    kmode = os.environ.get("KMODE", "split")
                                    if kmode == "act" or (
                                        kmode == "split"
                                        and j % ACT_TAKES_B_EVERY
                                        == ACT_TAKES_B_EVERY - 1
                                    ):
                                        nc.scalar.activation(
                                            out=u[1], in_=st[1][:, :], func=AF.Exp,
                                            scale=float(SCALE),
                                        )
                                    elif kmode == "dvecopy":
                                        nc.vector.tensor_copy(u[1], st[1][:, :])
                                    else:
                                        nc.vector._custom_dve(
                                            exp3, out=u[1], in0=st[1][:, :], in1=c3t,
                                            s0=c0, s1=c1, imm2=c2,
                                        )
                                else:
                                    u = None
                                if prev_u is not None:
                                    jm = prev_j
                                    for hp in range(4):
                                        us = prev_u[hp // 2][
                                            :, 512 * (hp % 2) : 512 * (hp % 2) + 512
                                        ]
                                        nc.tensor.matmul(
                                            av_ps[32 * hp : 32 * hp + 32, :],
                                            vp[:, jm,
                                               128 * g + 32 * hp : 128 * g + 32 * hp + 32],
                                            us,
                                            start=(jm == 0),
                                            stop=(jm == NJ - 1),
                                            tile_position=(0, 32 * hp),
                                        )
                                        nc.tensor.matmul(
                                            dn_ps[32 * hp : 32 * hp + 32, :],
                                            ones32[:, :],
                                            us,
                                            start=(jm == 0),
                                            stop=(jm == NJ - 1),
                                            tile_position=(0, 32 * hp),
                                        )
                                prev_u = u
                                prev_j = j
                            # normalize: attnT = av * (1/den) [+ bv]
                            rden = tails.tile([128, 512], F32, tag="rden")
                            nc.vector.reciprocal_approx_fast(rden, dn_ps[:, :])
                            dst = attnT[g][:, 512 * qb : 512 * qb + 512]
                            nc.vector.tensor_mul(dst, av_ps[:, :], rden)
                            if not trivial_affine:
                                nc.vector.tensor_scalar(
                                    out=dst, in0=dst, scalar1=vp_[g][:, 1:2],
                                    scalar2=None, op0=ALU.add,
                                )

                        # ---- tail for this q-block ----
                        for t in range(4):
                            y_ps = scp.tile([128, 1024], F32, tag="sc")
                            yp = y_ps[:, 0:256]
                            q0 = 512 * qb + 128 * t
                            for g in range(2):
                                nc.tensor.matmul(
                                    yp,
                                    attnT[g][:, q0 : q0 + 128],
                                    wot[g][:, :],
                                    start=(g == 0),
                                    stop=(g == 1),
                                )
                            if not trivial_affine:
                                nc.vector.tensor_add(yp, yp, vf_[:, 0, :])
                            # LN0 stats; rs = exp(-0.5*ln(var+eps))
                            st6 = tails.tile([128, 6], F32, tag="st6")
                            mv = tails.tile([128, 2], F32, tag="mv")
                            rs = tails.tile([128, 1], F32, tag="rs")
                            nc.vector.bn_stats(out=st6, in_=yp)
                            nc.vector.bn_aggr(out=mv, in_=st6)
                            nc.scalar.activation(
                                out=rs, in_=mv[:, 1:2], func=AF.Ln,
                                bias=epst[:, :],
                            )
                            nc.scalar.activation(
                                out=rs, in_=rs, func=AF.Exp, scale=-0.5
                            )
                            z = tails.tile([128, D], F32, tag="z")
                            if trivial_affine:
                                # z = t + relu(t), t = (y-mu)*rs -- one fused op
                                nc.vector._custom_dve(
                                    relu2, out=z, in0=yp, s0=mv[:, 0:1], s1=rs
                                )
                            else:
                                h0 = tails.tile([128, D], F32, tag="h0")
                                nc.vector.tensor_scalar(
                                    out=h0, in0=yp, scalar1=mv[:, 0:1],
                                    scalar2=rs, op0=ALU.subtract, op1=ALU.mult,
                                )
                                nc.vector.tensor_mul(h0, h0, vf_[:, 1, :])
                                nc.vector.tensor_add(h0, h0, vf_[:, 2, :])
                                zr = tails.tile([128, D], F32, tag="zr")
                                nc.vector.tensor_scalar_max(zr, h0, 0.0)
                                nc.vector.tensor_add(z, h0, zr)
                            # LN1
                            st6b = tails.tile([128, 6], F32, tag="st6b")
                            mvb = tails.tile([128, 2], F32, tag="mvb")
                            rsb = tails.tile([128, 1], F32, tag="rsb")
                            nc.vector.bn_stats(out=st6b, in_=z)
                            nc.vector.bn_aggr(out=mvb, in_=st6b)
                            nc.scalar.activation(
                                out=rsb, in_=mvb[:, 1:2], func=AF.Ln,
                                bias=epst[:, :],
                            )
                            nc.scalar.activation(
                                out=rsb, in_=rsb, func=AF.Exp, scale=-0.5
                            )
                            ot = tails.tile([128, D], F32, tag="ot")
                            nc.vector.tensor_scalar(
                                out=ot, in0=z, scalar1=mvb[:, 0:1], scalar2=rsb,
                                op0=ALU.subtract, op1=ALU.mult,
                            )
                            if not trivial_affine:
                                nc.vector.tensor_mul(ot, ot, vf_[:, 3, :])
                                nc.vector.tensor_add(ot, ot, vf_[:, 4, :])
                            nc.sync.dma_start(out=out[q0 : q0 + 128, :], in_=ot)

    nc.compile()
    return nc


_KERNEL_CACHE = {}


def _get_kernel(trivial_affine, repeat=1):
    key = (bool(trivial_affine), int(repeat), os.environ.get("KMODE", "split"))
    if key not in _KERNEL_CACHE:
        _KERNEL_CACHE[key] = _build_kernel(key[0], key[1])
    return _KERNEL_CACHE[key]


def _prep(Q, K, Wq, bq, Wk, bk, Wv, bv, Wo, bo, g0, beta0, g1, beta1):
    """Shared input prep: returns (trivial_affine, in_maps)."""
    Q = np.asarray(Q, dtype=np.float32)
    K = np.asarray(K, dtype=np.float32)
    Wq = np.asarray(Wq, dtype=np.float32)
    Wk = np.asarray(Wk, dtype=np.float32)
    Wv = np.asarray(Wv, dtype=np.float32)
    Wo = np.asarray(Wo, dtype=np.float32)
    bq, bv, bo, g0, beta0, g1, beta1 = [
        np.asarray(v, dtype=np.float32)
        for v in (bq, bv, bo, g0, beta0, g1, beta1)
    ]

    trivial = bool(
        not bq.any() and not bv.any() and not bo.any()
        and not beta0.any() and not beta1.any()
        and np.all(g0 == 1.0) and np.all(g1 == 1.0)
    )

    wqTn = np.ascontiguousarray(Wq.T)
    wkTn = np.ascontiguousarray(Wk.T)
    wvTn = np.ascontiguousarray(Wv.T)
    woTn = np.ascontiguousarray(Wo.T)
    vecsP = np.stack([bq, bv], axis=1).astype(np.float32)  # [D, 2]
    vecsF = np.stack([bo, g0, beta0, g1, beta1], axis=0).astype(np.float32)

    kTb = [np.ascontiguousarray(K[b].T) for b in range(B)]
    in_maps = []
    for c in range(NCORES):
        b, qc = divmod(c, NCORES // B)
        in_maps.append(
            {
                "qT": np.ascontiguousarray(Q[b, QC * qc : QC * qc + QC, :].T),
                "kT": kTb[b],
                "wqT": wqTn,
                "wkT": wkTn,
                "wvT": wvTn,
                "woT": woTn,
                "vecsP": vecsP,
                "vecsF": vecsF,
            }
        )
    return trivial, in_maps


def _gather(res):
    outp = np.empty((B, NQ, D), dtype=np.float32)
    for c in range(NCORES):
        b, qc = divmod(c, NCORES // B)
        outp[b, QC * qc : QC * qc + QC, :] = res.results[c]["out"]
    return outp


def kernel(**inputs):
    trivial, in_maps = _prep(**inputs)
    nc = _get_kernel(trivial)
    res = run_bass_kernel_spmd(nc, in_maps, list(range(NCORES)))
    return _gather(res)

